# revision 1
# baseline (speedup 1.0000x reference)
"""Causal multi-head attention block (B=8, T=2048, C=768, H=8) on 8 trn2 cores.

Sharding: data-parallel over batch — one batch element per NeuronCore, weights
replicated, no collectives.

Per-core algorithm (all matmuls float32r = full-speed ~fp32):
  Phase A: PE-transpose x_b -> x^T [c, t] in SBUF; w_attn/w_proj -> transposed
           DRAM scratch (watT, wpT).
  Phase B: per superblock of 4 heads: V = x @ w_v^T + b_v (natural [t, d]
           layout, with an appended ones column per head for the softmax
           denominator); per group of 2 heads: Q^T/K^T = (w x^T) + b (head-
           aligned [d, t] layout, Q pre-scaled by 1/sqrt(hs) via prescaled
           weights); per head: causal attention in S^T layout:
             S^T[j, i] = K^T.T-free matmul; P = exp(S^T) on ACT;
             diagonal-block mask multiply on DVE;
             O^T[d, i] (+ denominator row l) accumulated in PSUM over j-tiles
             via lhsT=[V|1]; normalize by 1/l (broadcast l across partitions
             with a selector matmul), spill O^T to DRAM.
  Phase C: out = sum_h O_h^T.T @ w_proj_h^T + b_proj, written [t, c].
"""

import math
import os
import sys
from contextlib import ExitStack

for _p in ("/opt/trn_rl_repo", "/root/.axon_site/_ro/trn_rl_repo"):
    if os.path.isdir(_p) and _p not in sys.path:
        sys.path.append(_p)

import numpy as np

import concourse.bass as bass  # noqa: F401  (import keeps bass registered)
from concourse import bacc
import concourse.mybir as mybir
import concourse.tile as tile
from concourse.bass_utils import run_bass_kernel_spmd

F32 = mybir.dt.float32
F32R = mybir.dt.float32r
EXP = mybir.ActivationFunctionType.Exp
ADD = mybir.AluOpType.add
MULT = mybir.AluOpType.mult

B, T, C, H, HS = 8, 2048, 768, 8, 96
KT = C // 128        # 6 contraction tiles of 128
TT = T // 128        # 16 t-tiles of 128
NCORES = 8


def _chunks(lo, hi, align=512):
    """Split [lo, hi) at multiples of `align`."""
    out = []
    a = lo
    while a < hi:
        b = min(hi, (a // align + 1) * align)
        out.append((a, b))
        a = b
    return out


def build_nc():
    nc = bacc.Bacc()
    x_b = nc.dram_tensor("x_b", [T, C], F32R, kind="ExternalInput")
    wat = nc.dram_tensor("wat", [3 * C, C], F32R, kind="ExternalInput")
    wp = nc.dram_tensor("wp", [C, C], F32R, kind="ExternalInput")
    ident = nc.dram_tensor("ident", [128, 128], F32R, kind="ExternalInput")
    mk = nc.dram_tensor("mk", [128, 128], F32R, kind="ExternalInput")
    bsel = nc.dram_tensor("bsel", [128, HS], F32R, kind="ExternalInput")
    bqk = nc.dram_tensor("bqk", [HS, 16], F32, kind="ExternalInput")
    bv = nc.dram_tensor("bv", [128, C], F32, kind="ExternalInput")
    bo = nc.dram_tensor("bo", [128, C], F32, kind="ExternalInput")
    out = nc.dram_tensor("out", [T, C], F32, kind="ExternalOutput")

    with tile.TileContext(nc) as tc, ExitStack() as ctx:
        dram = ctx.enter_context(tc.tile_pool(name="dram", bufs=1, space="DRAM"))
        wpT = dram.tile([C, C], F32R, tag="wpT")
        oT = dram.tile([H, HS, T], F32R, tag="oT")

        consts = ctx.enter_context(tc.tile_pool(name="consts", bufs=1))
        id_sb = consts.tile([128, 128], F32R, tag="id")
        mk_sb = consts.tile([128, 128], F32R, tag="mk")
        bs_sb = consts.tile([128, HS], F32R, tag="bs")
        bqk_sb = consts.tile([HS, 16], F32, tag="bqk")
        bv_sb = consts.tile([128, C], F32, tag="bv")
        bo_sb = consts.tile([128, C], F32, tag="bo")
        one_f32 = consts.tile([128, 1], F32, tag="one")
        nc.vector.memset(one_f32[:], 1.0)
        # ident first: the very first transposes need it; other consts follow
        # the big phase-A loads so they don't delay the critical path
        nc.sync.dma_start(id_sb[:], ident[:, :])

        xTp = ctx.enter_context(tc.tile_pool(name="xT", bufs=1))
        xT = xTp.tile([128, KT, T], F32R, tag="xT")
        waTp = ctx.enter_context(tc.tile_pool(name="waT", bufs=1))
        waT = waTp.tile([128, KT, 3 * C], F32R, tag="waT")   # w_attn^T resident

        # ---------------- Phase A: transposes ----------------
        # One fat DMA per source tensor, transposed-row accumulation in SBUF,
        # one fat DMA per 128-row stripe of the transposed weights. This keeps
        # the DMA instruction count low (each dma_start costs ~625ns of shared
        # HWDGE time regardless of size).
        with tc.tile_pool(name="pa_in", bufs=3) as pin, \
             tc.tile_pool(name="pa_ps", bufs=4, space="PSUM") as pps, \
             tc.tile_pool(name="pa_ev", bufs=2) as pev:
            # w_attn first (its transpose feeds the projections): the V-head
            # stripes, then interleaved q/k stripes so early heads unblock
            # first; x after; w_proj last (only needed in phase C).
            wat_r = wat.rearrange("(a p) c -> p a c", p=128)
            x_r = x_b.rearrange("(a p) c -> p a c", p=128)

            def tr_group(src, sls, kc, dst):
                """Transpose len(sls) consecutive 128-blocks of `src` stripes
                into one wide PSUM tile, then evict with a single copy."""
                n = len(sls)
                psb = pps.tile([128, n * 128], F32R, tag="tps")
                for i, sl in enumerate(sls):
                    rgn0 = (i * 128) // 512
                    rgn1 = ((i + 1) * 128 - 1) // 512
                    first = (i == 0) or rgn0 != ((i * 128 - 1) // 512)
                    last = (i == n - 1) or rgn1 != (((i + 2) * 128 - 1) // 512)
                    nc.tensor.matmul(psb[:, i * 128:(i + 1) * 128],
                                     src[:, sl, kc * 128:(kc + 1) * 128],
                                     id_sb[:], is_transpose=True,
                                     start=first, stop=last)
                nc.any.tensor_copy(dst, psb[:])

            def w_stripes(tile_, rt_groups):
                for kc in range(KT):
                    for (sl0, rt0, n) in rt_groups:
                        tr_group(tile_, range(sl0, sl0 + n), kc,
                                 waT[:, kc, rt0 * 128:(rt0 + n) * 128])

            vch = pin.tile([128, 6, C], F32R, tag="ain", name="vch")
            nc.sync.dma_start(vch[:], wat_r[:, 12:18, :])
            nc.sync.dma_start(mk_sb[:], mk[:, :])
            nc.sync.dma_start(bs_sb[:], bsel[:, :])
            nc.sync.dma_start(bqk_sb[:], bqk[:, :])
            nc.sync.dma_start(bv_sb[:], bv[:, :])
            nc.sync.dma_start(bo_sb[:], bo[:, :])
            w_stripes(vch, [(0, 12, 6)])

            for ch in range(3):
                xch = pin.tile([128, 6, C], F32R, tag="ain", name=f"xch{ch}")
                nch = 6 if ch < 2 else 4
                nc.sync.dma_start(xch[:, 0:nch, :],
                                  x_r[:, ch * 6:ch * 6 + nch, :])
                for kc in range(KT):
                    tr_group(xch, range(nch), kc,
                             xT[:, kc, ch * 6 * 128:(ch * 6 + nch) * 128])

            qk0 = pin.tile([128, 6, C], F32R, tag="ain", name="qk0")
            nc.sync.dma_start(qk0[:, 0:3, :], wat_r[:, 0:3, :])
            nc.sync.dma_start(qk0[:, 3:6, :], wat_r[:, 6:9, :])
            w_stripes(qk0, [(0, 0, 3), (3, 6, 3)])
            qk1 = pin.tile([128, 6, C], F32R, tag="ain", name="qk1")
            nc.sync.dma_start(qk1[:, 0:3, :], wat_r[:, 3:6, :])
            nc.sync.dma_start(qk1[:, 3:6, :], wat_r[:, 9:12, :])
            w_stripes(qk1, [(0, 3, 3), (3, 9, 3)])
            wpin = pin.tile([128, 6, C], F32R, tag="ain", name="wpin")
            nc.sync.dma_start(wpin[:], wp.rearrange("(a p) c -> p a c", p=128))
            for kc in range(KT):
                row = pev.tile([128, C], F32R, tag="wrow")
                tr_group(wpin, range(C // 128), kc, row[:])
                nc.gpsimd.dma_start(wpT[kc * 128:(kc + 1) * 128, :], row[:])

        # ---------------- Phase B: projections + attention ----------------
        with tc.tile_pool(name="vsb", bufs=2) as vsbp, \
             tc.tile_pool(name="qk", bufs=4) as qkp, \
             tc.tile_pool(name="pt", bufs=2) as ptp, \
             tc.tile_pool(name="ep", bufs=2) as epp, \
             tc.tile_pool(name="bps", bufs=2, space="PSUM") as bps, \
             tc.tile_pool(name="pj", bufs=2, space="PSUM") as pjps, \
             tc.tile_pool(name="ops", bufs=1, space="PSUM") as opsp:
            for pr in range(4):
                # V projection for this pair of heads, natural [t, d] layout
                # with an appended ones column per head (softmax denominator).
                # The matmul N is padded to 256 (f32r needs free >= 256 for
                # full rate); the pad may read into neighbouring v columns.
                start_off = min(2 * HS * pr, C - 256)
                off = 2 * HS * pr - start_off
                V = vsbp.tile([128, TT, 2, HS + 1], F32R, tag="V")
                nc.vector.tensor_copy(
                    V.rearrange("p a b c -> p (a b c)"),
                    one_f32[:].to_broadcast([128, TT * 2 * (HS + 1)]))
                for tt in range(TT):
                    vps = pjps.tile([128, 512], F32, tag="pj")
                    for kc in range(KT):
                        nc.tensor.matmul(vps[:, 0:256],
                                         xT[:, kc, tt * 128:(tt + 1) * 128],
                                         waT[:, kc, 2 * C + start_off:
                                             2 * C + start_off + 256],
                                         start=(kc == 0), stop=(kc == KT - 1))
                    nc.vector.tensor_tensor(
                        V[:, tt, :, 0:HS],
                        vps[:, off:off + 2 * HS]
                            .rearrange("p (h d) -> p h d", d=HS),
                        bv_sb[:, 2 * HS * pr:2 * HS * (pr + 1)]
                            .rearrange("p (h d) -> p h d", d=HS),
                        ADD)

                for hh in range(2):
                    h = 2 * pr + hh
                    # Q^T/K^T projection for head h ([d, t] layout); per-head
                    # granularity lets the next head's projection overlap the
                    # current head's (ACT-bound) attention inner loop.
                    qkh = [qkp.tile([128, T], F32R, tag="qk", name=f"qk{i}")
                           for i in range(2)]
                    for tc4 in range(4):
                        for mc in range(2):          # 0 = q, 1 = k
                            wc = h * HS + (0 if mc == 0 else C)
                            pj = pjps.tile([128, 512], F32, tag="pj")
                            for kc in range(KT):
                                nc.tensor.matmul(
                                    pj[0:HS, 0:512],
                                    waT[:, kc, wc:wc + HS],
                                    xT[:, kc, tc4 * 512:(tc4 + 1) * 512],
                                    start=(kc == 0), stop=(kc == KT - 1))
                            m_col = h + (0 if mc == 0 else 8)
                            nc.vector.tensor_tensor(
                                qkh[mc][0:HS, tc4 * 512:(tc4 + 1) * 512],
                                pj[0:HS, 0:512],
                                bqk_sb[:, m_col:m_col + 1].to_broadcast([HS, 512]),
                                ADD)

                    qT, kT = qkh[0], qkh[1]
                    Oe = epp.tile([HS, T], F32R, tag="Oe", bufs=1)
                    for ihalf in range(2):
                        ibase = 1024 * ihalf
                        iend = ibase + 1024
                        njt = 8 * (ihalf + 1)
                        O_ps = opsp.tile([128, 1024], F32, tag="O")
                        for jt in range(njt):
                            j0 = 128 * jt
                            i0 = max(j0, ibase)
                            ilen = iend - i0
                            S = bps.tile([128, 1024], F32, tag="ps")
                            for (ra, rb) in _chunks(0, ilen):
                                nc.tensor.matmul(S[:, ra:rb],
                                                 kT[0:HS, j0:j0 + 128],
                                                 qT[0:HS, i0 + ra:i0 + rb],
                                                 start=True, stop=True)
                            P = ptp.tile([128, 1024], F32R, tag="P")
                            nc.scalar.activation(P[:, 0:ilen], S[:, 0:ilen],
                                                 EXP)
                            if j0 >= ibase:
                                nc.gpsimd.tensor_tensor(P[:, 0:128],
                                                        P[:, 0:128],
                                                        mk_sb[:], MULT)
                            for (a, b) in _chunks(i0, iend):
                                ci = a // 512
                                last_jt = min(4 * ci + 3, njt - 1)
                                nc.tensor.matmul(
                                    O_ps[0:HS + 1, a - ibase:b - ibase],
                                    V[:, jt, hh, :],
                                    P[:, a - i0:b - i0],
                                    start=(jt == 0), stop=(jt == last_jt))
                        # epilogue: normalize by the denominator row
                        lt = epp.tile([HS + 1, 1024], F32R, tag="lt",
                                      bufs=1)
                        nc.vector.tensor_copy(lt[:], O_ps[0:HS + 1, :])
                        Lp = bps.tile([128, 1024], F32, tag="ps")
                        for (ra, rb) in ((0, 512), (512, 1024)):
                            nc.tensor.matmul(Lp[0:HS, ra:rb],
                                             bs_sb[0:HS + 1, :],
                                             lt[:, ra:rb],
                                             start=True, stop=True)
                        R = epp.tile([HS, 1024], F32, tag="R", bufs=1)
                        nc.vector.reciprocal(R[:], Lp[0:HS, :])
                        nc.gpsimd.tensor_tensor(Oe[:, ibase:iend],
                                                lt[0:HS, :], R[:], MULT)
                    nc.gpsimd.dma_start(oT[h, :, :], Oe[:])

        # ---------------- Phase C: output projection ----------------
        # O^T streamed back from DRAM per 2-t-tile block (pipelined), K=96
        # contraction (no partition padding needed).
        oT_r = oT.rearrange("h p n -> p h n")
        with tc.tile_pool(name="pc", bufs=2) as pcp, \
             tc.tile_pool(name="pcb", bufs=1) as pcb, \
             tc.tile_pool(name="pco", bufs=3) as pco, \
             tc.tile_pool(name="pc_ps", bufs=2, space="PSUM") as pcps:
            wpT_sb = pcb.tile([HS, H, C], F32R, tag="wpTsb")
            nc.sync.dma_start(wpT_sb[:],
                              wpT.rearrange("(h p) n -> p h n", p=HS))
            out_r = out.rearrange("(g a p) c -> p g a c", a=2, p=128)
            for tg in range(TT // 2):
                otg = pco.tile([HS, H, 256], F32R, tag="otg")
                # heads 0..6 prefetch during head 7's attention; the head-7
                # slice is the only piece on the tail critical path
                nc.sync.dma_start(otg[:, 0:H - 1, :],
                                  oT_r[:, 0:H - 1, tg * 256:(tg + 1) * 256])
                nc.sync.dma_start(otg[:, H - 1:H, :],
                                  oT_r[:, H - 1:H, tg * 256:(tg + 1) * 256])
                o_sb = pcp.tile([128, 2, C], F32, tag="osb")
                for ta in range(2):
                    cps = pcps.tile([128, 1024], F32, tag="cps")
                    for (a, b) in ((0, 512), (512, C)):
                        for h in range(H):
                            nc.tensor.matmul(cps[:, a:b],
                                             otg[:, h, ta * 128:(ta + 1) * 128],
                                             wpT_sb[:, h, a:b],
                                             start=(h == 0), stop=(h == H - 1))
                    nc.vector.tensor_tensor(o_sb[:, ta, :], cps[:, 0:C],
                                            bo_sb[:], ADD)
                nc.gpsimd.dma_start(out_r[:, tg], o_sb[:])

    nc.finalize()
    return nc


_NC_CACHE = {}


def _get_nc():
    if "nc" not in _NC_CACHE:
        _NC_CACHE["nc"] = build_nc()
    return _NC_CACHE["nc"]


def _make_consts(b_attn, b_proj):
    s = 1.0 / math.sqrt(HS)
    bqk = np.empty((HS, 16), dtype=np.float32)
    for m in range(8):
        bqk[:, m] = b_attn[m * HS:(m + 1) * HS] * s
    for m in range(8):
        bqk[:, 8 + m] = b_attn[C + m * HS:C + (m + 1) * HS]
    bv = np.ascontiguousarray(
        np.broadcast_to(b_attn[2 * C:3 * C], (128, C)).astype(np.float32))
    bo = np.ascontiguousarray(
        np.broadcast_to(b_proj, (128, C)).astype(np.float32))
    ident = np.eye(128, dtype=np.float32)
    mk = np.triu(np.ones((128, 128), dtype=np.float32))
    bsel = np.zeros((128, HS), dtype=np.float32)
    bsel[HS, :] = 1.0
    return bqk, bv, bo, ident, mk, bsel


def kernel(x, w_attn, b_attn, w_proj, b_proj, _want_results=False, **run_kwargs):
    x = np.asarray(x, dtype=np.float32)
    w_attn = np.asarray(w_attn, dtype=np.float32)
    b_attn = np.asarray(b_attn, dtype=np.float32)
    w_proj = np.asarray(w_proj, dtype=np.float32)
    b_proj = np.asarray(b_proj, dtype=np.float32)

    s = 1.0 / math.sqrt(HS)
    wat = w_attn.copy()
    wat[0:C, :] *= s            # fold the 1/sqrt(hs) logit scale into Q
    bqk, bv, bo, ident, mk, bsel = _make_consts(b_attn, b_proj)

    nc = _get_nc()
    common = dict(wat=wat, wp=w_proj, ident=ident, mk=mk, bsel=bsel,
                  bqk=bqk, bv=bv, bo=bo)
    in_maps = [dict(x_b=np.ascontiguousarray(x[c]), **common)
               for c in range(NCORES)]
    res = run_bass_kernel_spmd(nc, in_maps, core_ids=list(range(NCORES)),
                               **run_kwargs)
    out = np.stack([res.results[c]["out"] for c in range(NCORES)], axis=0)
    if _want_results:
        return out, res
    return out


if __name__ == "__main__":
    rng = np.random.default_rng(0)
    x = rng.standard_normal((B, T, C), dtype=np.float32)
    w_attn = rng.standard_normal((3 * C, C), dtype=np.float32) / math.sqrt(C)
    b_attn = rng.standard_normal(3 * C).astype(np.float32) * 0.02
    w_proj = rng.standard_normal((C, C), dtype=np.float32) / math.sqrt(C)
    b_proj = rng.standard_normal(C).astype(np.float32) * 0.02
    o = kernel(x, w_attn, b_attn, w_proj, b_proj)
    print("out", o.shape, o.dtype, float(np.abs(o).mean()))



# revision 3
# speedup vs baseline: 1.4513x; 1.4513x over previous
"""Causal multi-head attention block (B=8, T=2048, C=768, H=8) on 8 trn2 cores.

Sharding: data-parallel over batch — one batch element per NeuronCore, weights
replicated, no collectives.

Host prep: x, w_attn, w_proj are pre-transposed on the host (numpy), the
1/sqrt(hs) logit scale is folded into the Q weights/bias, and w_proj^T is cast
to bf16 — so the device kernel has no transpose phase at all.

Per-core algorithm:
  Load x^T / w^T slices straight into SBUF (weights streamed per-head).
  V = x @ w_v^T + b_v for all 8 heads in natural [t, d] layout (bf16, with a
  ones column per head for the softmax denominator).
  Per head: Q^T/K^T = w x^T + b in head-aligned [d, t] layout (f32r);
  causal attention in S^T layout: S^T[j, i] matmul -> P = exp(S^T) on ACT
  (bf16 out) -> diagonal-block mask multiply on gpsimd -> O^T[d, i] (+
  denominator row l) accumulated in PSUM over j-tiles via lhsT=[V|1] (bf16);
  epilogue: broadcast l across partitions with a selector matmul, fast
  approximate reciprocal on DVE, normalize on gpsimd -> bf16 O^T resident in
  SBUF (no DRAM spill).
  Output projection from SBUF per t-tile: out = sum_h O_h^T.T @ w_proj_h^T
  + b_proj, written [t, c].
"""

import math
import os
import sys
from contextlib import ExitStack

for _p in ("/opt/trn_rl_repo", "/root/.axon_site/_ro/trn_rl_repo"):
    if os.path.isdir(_p) and _p not in sys.path:
        sys.path.append(_p)

import numpy as np
import ml_dtypes

import concourse.bass as bass  # noqa: F401  (import keeps bass registered)
from concourse import bacc
import concourse.mybir as mybir
import concourse.tile as tile
from concourse.bass_utils import run_bass_kernel_spmd

F32 = mybir.dt.float32
F32R = mybir.dt.float32r
BF16 = mybir.dt.bfloat16
EXP = mybir.ActivationFunctionType.Exp
ADD = mybir.AluOpType.add
MULT = mybir.AluOpType.mult

B, T, C, H, HS = 8, 2048, 768, 8, 96
KT = C // 128        # 6 contraction tiles of 128
TT = T // 128        # 16 t-tiles of 128
NCORES = 8
BF16_NP = ml_dtypes.bfloat16


def _chunks(lo, hi, align=512):
    """Split [lo, hi) at multiples of `align`."""
    out = []
    a = lo
    while a < hi:
        b = min(hi, (a // align + 1) * align)
        out.append((a, b))
        a = b
    return out


def build_nc():
    nc = bacc.Bacc()
    xT_d = nc.dram_tensor("xT", [C, T], F32R, kind="ExternalInput")
    wqk_d = nc.dram_tensor("wqk", [C, H, 2 * HS], F32R, kind="ExternalInput")
    wv_d = nc.dram_tensor("wv", [C, C], F32R, kind="ExternalInput")
    wpT_d = nc.dram_tensor("wpT", [C, C], BF16, kind="ExternalInput")
    mk_d = nc.dram_tensor("mk", [128, 128], BF16, kind="ExternalInput")
    bsel_d = nc.dram_tensor("bsel", [HS + 1, HS], F32R, kind="ExternalInput")
    bqk_d = nc.dram_tensor("bqk", [HS, 16], F32, kind="ExternalInput")
    bv_d = nc.dram_tensor("bv", [128, C], F32, kind="ExternalInput")
    bo_d = nc.dram_tensor("bo", [128, C], F32, kind="ExternalInput")
    out = nc.dram_tensor("out", [T, C], F32, kind="ExternalOutput")

    xT_r = xT_d.rearrange("(k p) t -> p k t", p=128)
    wqk_r = wqk_d.rearrange("(k p) h c -> p k h c", p=128)
    wv_r = wv_d.rearrange("(k p) c -> p k c", p=128)
    wpT_r = wpT_d.rearrange("(h p) c -> p h c", p=HS)
    out_r = out.rearrange("(a p) c -> p a c", p=128)

    with tile.TileContext(nc) as tc, ExitStack() as ctx:
        consts = ctx.enter_context(tc.tile_pool(name="consts", bufs=1))
        mk_sb = consts.tile([128, 128], BF16, tag="mk")
        bs_sb = consts.tile([HS + 1, HS], F32R, tag="bs")
        bqk_sb = consts.tile([HS, 16], F32, tag="bqk")
        bv_sb = consts.tile([128, C], F32, tag="bv")
        bo_sb = consts.tile([128, C], F32, tag="bo")
        wpT_sb = consts.tile([HS, H, C], BF16, tag="wpT")
        one_bf = consts.tile([128, 1], BF16, tag="one")
        nc.vector.memset(one_bf[:], 1.0)

        xTp = ctx.enter_context(tc.tile_pool(name="xT", bufs=1))
        xT = xTp.tile([128, KT, T], F32R, tag="xT")
        Vp = ctx.enter_context(tc.tile_pool(name="V", bufs=1))
        V = Vp.tile([128, TT, H, HS + 1], BF16, tag="V")
        oep = ctx.enter_context(tc.tile_pool(name="oe", bufs=1))
        Oe = [oep.tile([HS, T], BF16, tag=f"oe{h}", name=f"oe{h}")
              for h in range(H)]

        with tc.tile_pool(name="wv", bufs=2) as wvp, \
             tc.tile_pool(name="wqk", bufs=2) as wqkp, \
             tc.tile_pool(name="qk", bufs=4) as qkp, \
             tc.tile_pool(name="pt", bufs=2) as ptp, \
             tc.tile_pool(name="lt", bufs=2) as ltp, \
             tc.tile_pool(name="rc", bufs=1) as rcp, \
             tc.tile_pool(name="ob", bufs=3) as obp, \
             tc.tile_pool(name="pj", bufs=2, space="PSUM") as pjp, \
             tc.tile_pool(name="sp", bufs=2, space="PSUM") as spp, \
             tc.tile_pool(name="op", bufs=2, space="PSUM") as opp:

            # ---- input DMAs, ordered for the pipeline ----
            # small consts + the first weight slices first, then x^T in
            # 512-column chunks so V/QK projections start early.
            nc.sync.dma_start(mk_sb[:], mk_d[:, :])
            nc.sync.dma_start(bs_sb[:], bsel_d[:, :])
            nc.sync.dma_start(bqk_sb[:], bqk_d[:, :])
            nc.sync.dma_start(bv_sb[:], bv_d[:, :])
            wv_sb = [wvp.tile([128, KT, 4 * HS], F32R, tag="wv",
                              name=f"wv{g}") for g in range(2)]
            nc.sync.dma_start(wv_sb[0][:], wv_r[:, :, 0:4 * HS])
            wqk_sb = {0: wqkp.tile([128, KT, 2 * HS], F32R, tag="wqk",
                                   name="wqk0")}
            nc.sync.dma_start(wqk_sb[0][:], wqk_r[:, :, 0, :])
            for chs, che in ((0, 512), (512, 1024), (1024, 1536), (1536, 2048)):
                nc.sync.dma_start(xT[:, :, chs:che], xT_r[:, :, chs:che])
            nc.sync.dma_start(wv_sb[1][:], wv_r[:, :, 4 * HS:8 * HS])
            wqk_sb[1] = wqkp.tile([128, KT, 2 * HS], F32R, tag="wqk",
                                  name="wqk1")
            nc.sync.dma_start(wqk_sb[1][:], wqk_r[:, :, 1, :])
            nc.sync.dma_start(bo_sb[:], bo_d[:, :])
            nc.sync.dma_start(wpT_sb[:], wpT_r[:, :, :])

            # ones columns for the softmax denominator
            nc.vector.tensor_copy(V[:, :, :, HS:HS + 1],
                                  one_bf[:].to_broadcast([128, TT, H, 1]))

            def qk_proj_chunk(h, tc4):
                """Q^T/K^T projection for head h, t-columns [512*tc4, +512)."""
                for mc in range(2):          # 0 = q, 1 = k
                    pj = pjp.tile([128, 512], F32, tag="pj", name="pj")
                    for kc in range(KT):
                        nc.tensor.matmul(
                            pj[0:HS, 0:512],
                            wqk_sb[h][:, kc, mc * HS:(mc + 1) * HS],
                            xT[:, kc, tc4 * 512:(tc4 + 1) * 512],
                            start=(kc == 0), stop=(kc == KT - 1))
                    m_col = h + (0 if mc == 0 else 8)
                    nc.vector.tensor_tensor(
                        qkh[h][mc][0:HS, tc4 * 512:(tc4 + 1) * 512],
                        pj[0:HS, 0:512],
                        bqk_sb[:, m_col:m_col + 1].to_broadcast([HS, 512]),
                        ADD)

            # qk tiles for head h and h+1 live concurrently (pool bufs=4)
            qkh = {}

            def alloc_qk(h):
                qkh[h] = [qkp.tile([HS, T], F32R, tag="qk", name=f"qk{h}_{i}")
                          for i in range(2)]

            # ---- V projection (all heads) + head-0 QK proj, pipelined with
            # the x^T chunk DMAs ----
            alloc_qk(0)
            for ch in range(4):
                for g in range(2):
                    for tt in range(4 * ch, 4 * ch + 4):
                        vps = pjp.tile([128, 512], F32, tag="pj", name="vps")
                        for kc in range(KT):
                            nc.tensor.matmul(
                                vps[:, 0:4 * HS],
                                xT[:, kc, tt * 128:(tt + 1) * 128],
                                wv_sb[g][:, kc, :],
                                start=(kc == 0), stop=(kc == KT - 1))
                        nc.vector.tensor_tensor(
                            V[:, tt, 4 * g:4 * g + 4, 0:HS],
                            vps[:, 0:4 * HS].rearrange("p (h d) -> p h d",
                                                       d=HS),
                            bv_sb[:, 4 * HS * g:4 * HS * (g + 1)]
                                .rearrange("p (h d) -> p h d", d=HS),
                            ADD)
                qk_proj_chunk(0, ch)

            # ---- per-head attention ----
            for h in range(H):
                if h + 2 < H:
                    wqk_sb[h + 2] = wqkp.tile([128, KT, 2 * HS], F32R,
                                              tag="wqk", name=f"wqk{h + 2}")
                    nc.sync.dma_start(wqk_sb[h + 2][:], wqk_r[:, :, h + 2, :])

                qT, kT = qkh[h]
                for ihalf in range(2):
                    ibase = 1024 * ihalf
                    iend = ibase + 1024
                    njt = 8 * (ihalf + 1)
                    O_ps = opp.tile([128, 1024], F32, tag="O", name="O_ps")
                    for jt in range(njt):
                        j0 = 128 * jt
                        i0 = max(j0, ibase)
                        ilen = iend - i0
                        P = ptp.tile([128, 1024], BF16, tag="P", name="P")
                        for (ra, rb) in _chunks(0, ilen):
                            S = spp.tile([128, 512], F32, tag="S", name="S")
                            nc.tensor.matmul(S[:, 0:rb - ra],
                                             kT[0:HS, j0:j0 + 128],
                                             qT[0:HS, i0 + ra:i0 + rb],
                                             start=True, stop=True)
                            nc.scalar.activation(P[:, ra:rb], S[:, 0:rb - ra],
                                                 EXP)
                        if j0 >= ibase:
                            nc.gpsimd.tensor_tensor(P[:, 0:128],
                                                    P[:, 0:128],
                                                    mk_sb[:], MULT)
                        for (a, b) in _chunks(i0, iend):
                            ci = a // 512
                            last_jt = min(4 * ci + 3, njt - 1)
                            nc.tensor.matmul(
                                O_ps[0:HS + 1, a - ibase:b - ibase],
                                V[:, jt, h, :],
                                P[:, a - i0:b - i0],
                                start=(jt == 0), stop=(jt == last_jt))
                    # epilogue: normalize by the denominator row (l is row HS
                    # of O_ps).  lt copy on ACT (Copy is in the Exp table, no
                    # reload); broadcast l across partitions with a selector
                    # matmul; fast approximate reciprocal on DVE.
                    lt = ltp.tile([HS + 1, 1024], F32R, tag="lt", name="lt")
                    nc.scalar.activation(lt[:], O_ps[0:HS + 1, :],
                                         mybir.ActivationFunctionType.Copy)
                    Lp = opp.tile([128, 1024], F32, tag="O", name="Lp")
                    R = rcp.tile([HS, 1024], F32, tag="R", name="R")
                    for (ra, rb) in ((0, 512), (512, 1024)):
                        nc.tensor.matmul(Lp[0:HS, ra:rb],
                                         bs_sb[:, :],
                                         lt[:, ra:rb],
                                         start=True, stop=True)
                        nc.vector.reciprocal_approx_fast(R[:, ra:rb],
                                                         Lp[0:HS, ra:rb])
                    nc.gpsimd.tensor_tensor(Oe[h][:, ibase:iend],
                                            lt[0:HS, :], R[:], MULT)

                if h + 1 < H:
                    alloc_qk(h + 1)
                    for tc4 in range(4):
                        qk_proj_chunk(h + 1, tc4)

            # ---- output projection, from SBUF ----
            for tt in range(TT):
                cps = opp.tile([128, 1024], F32, tag="O", name="cps")
                for (a, b) in ((0, 512), (512, C)):
                    for h in range(H):
                        nc.tensor.matmul(cps[:, a:b],
                                         Oe[h][:, tt * 128:(tt + 1) * 128],
                                         wpT_sb[:, h, a:b],
                                         start=(h == 0), stop=(h == H - 1))
                o_sb = obp.tile([128, C], F32, tag="o", name="o_sb")
                nc.vector.tensor_tensor(o_sb[:], cps[:, 0:C], bo_sb[:], ADD)
                nc.gpsimd.dma_start(out_r[:, tt], o_sb[:])

    nc.finalize()
    return nc


_NC_CACHE = {}


def _get_nc():
    if "nc" not in _NC_CACHE:
        _NC_CACHE["nc"] = build_nc()
    return _NC_CACHE["nc"]


def _make_consts(w_attn, b_attn, w_proj, b_proj):
    s = 1.0 / math.sqrt(HS)
    waT = np.ascontiguousarray(w_attn.T)          # [C, 3C]
    wqk = np.empty((C, H, 2 * HS), dtype=np.float32)
    for h in range(H):
        wqk[:, h, 0:HS] = waT[:, h * HS:(h + 1) * HS] * s
        wqk[:, h, HS:2 * HS] = waT[:, C + h * HS:C + (h + 1) * HS]
    wv = np.ascontiguousarray(waT[:, 2 * C:3 * C])
    wpT = np.ascontiguousarray(w_proj.T).astype(BF16_NP)
    bqk = np.empty((HS, 16), dtype=np.float32)
    for m in range(8):
        bqk[:, m] = b_attn[m * HS:(m + 1) * HS] * s
    for m in range(8):
        bqk[:, 8 + m] = b_attn[C + m * HS:C + (m + 1) * HS]
    bv = np.ascontiguousarray(
        np.broadcast_to(b_attn[2 * C:3 * C], (128, C)).astype(np.float32))
    bo = np.ascontiguousarray(
        np.broadcast_to(b_proj, (128, C)).astype(np.float32))
    mk = np.triu(np.ones((128, 128), dtype=np.float32)).astype(BF16_NP)
    bsel = np.zeros((HS + 1, HS), dtype=np.float32)
    bsel[HS, :] = 1.0
    return wqk, wv, wpT, bqk, bv, bo, mk, bsel


def kernel(x, w_attn, b_attn, w_proj, b_proj, _want_results=False, **run_kwargs):
    x = np.asarray(x, dtype=np.float32)
    w_attn = np.asarray(w_attn, dtype=np.float32)
    b_attn = np.asarray(b_attn, dtype=np.float32)
    w_proj = np.asarray(w_proj, dtype=np.float32)
    b_proj = np.asarray(b_proj, dtype=np.float32)

    wqk, wv, wpT, bqk, bv, bo, mk, bsel = _make_consts(
        w_attn, b_attn, w_proj, b_proj)

    nc = _get_nc()
    common = dict(wqk=wqk, wv=wv, wpT=wpT, bqk=bqk, bv=bv, bo=bo, mk=mk,
                  bsel=bsel)
    in_maps = [dict(xT=np.ascontiguousarray(x[c].T), **common)
               for c in range(NCORES)]
    res = run_bass_kernel_spmd(nc, in_maps, core_ids=list(range(NCORES)),
                               **run_kwargs)
    out = np.stack([res.results[c]["out"] for c in range(NCORES)], axis=0)
    if _want_results:
        return out, res
    return out


if __name__ == "__main__":
    rng = np.random.default_rng(0)
    x = rng.standard_normal((B, T, C), dtype=np.float32)
    w_attn = rng.standard_normal((3 * C, C), dtype=np.float32) / math.sqrt(C)
    b_attn = rng.standard_normal(3 * C).astype(np.float32) * 0.02
    w_proj = rng.standard_normal((C, C), dtype=np.float32) / math.sqrt(C)
    b_proj = rng.standard_normal(C).astype(np.float32) * 0.02
    o = kernel(x, w_attn, b_attn, w_proj, b_proj)
    print("out", o.shape, o.dtype, float(np.abs(o).mean()))


# revision 7
# speedup vs baseline: 1.5625x; 1.0766x over previous
"""Causal multi-head attention block (B=8, T=2048, C=768, H=8) on 8 trn2 cores.

Sharding: data-parallel over batch — one batch element per NeuronCore, weights
replicated, no collectives.

Host prep: x, w_attn, w_proj are pre-transposed on the host (numpy), the
1/sqrt(hs) logit scale is folded into the Q weights/bias, and w_proj^T is cast
to bf16 — so the device kernel has no transpose phase at all.

Per-core algorithm:
  Load x^T / w^T slices straight into SBUF (weights streamed per-head).
  V = x @ w_v^T + b_v for all 8 heads in natural [t, d] layout (bf16, with a
  ones column per head for the softmax denominator).
  Per head: Q^T/K^T = w x^T + b in head-aligned [d, t] layout (bf16);
  causal attention in S^T layout: S^T[j, i] matmul (f32 PSUM) -> P = exp(S^T)
  on ACT (bf16 out) -> diagonal-block mask multiply on gpsimd -> O^T[d, i]
  (+ denominator row l) accumulated in PSUM over j-tiles via lhsT=[V|1]
  (bf16).  Epilogue (emission deferred so it never head-of-line-blocks the
  tensor queue): copy O_ps to SBUF on gpsimd, broadcast l across partitions
  with a selector matmul, fast approximate reciprocal on DVE, then normalize
  on gpsimd straight into a 128-row-aligned resident O^T stripe tile (bf16,
  no DRAM spill).
  Output projection from SBUF per t-tile with K=128 stripe contraction:
  out = O^T.T @ w_proj^T + b_proj, written [t, c].
"""

import math
import os
import sys
from contextlib import ExitStack

for _p in ("/opt/trn_rl_repo", "/root/.axon_site/_ro/trn_rl_repo"):
    if os.path.isdir(_p) and _p not in sys.path:
        sys.path.append(_p)

import numpy as np
import ml_dtypes

import concourse.bass as bass  # noqa: F401  (import keeps bass registered)
from concourse import bacc
import concourse.mybir as mybir
import concourse.tile as tile
from concourse.bass_utils import run_bass_kernel_spmd

F32 = mybir.dt.float32
F32R = mybir.dt.float32r
BF16 = mybir.dt.bfloat16
EXP = mybir.ActivationFunctionType.Exp
ADD = mybir.AluOpType.add
MULT = mybir.AluOpType.mult

B, T, C, H, HS = 8, 2048, 768, 8, 96
KT = C // 128        # 6 contraction tiles of 128
TT = T // 128        # 16 t-tiles of 128
NCORES = 8
BF16_NP = ml_dtypes.bfloat16

# head h occupies rows 96h..96h+95 of the packed [768, T] O^T; as 6 stripes of
# 128 partitions each head maps to 1-2 (stripe, row0, d0, length) segments
_OSEGS = []
for _h in range(H):
    g0 = _h * HS
    s0, r0 = g0 // 128, g0 % 128
    if r0 + HS <= 128:
        _OSEGS.append([(s0, r0, 0, HS)])
    else:
        n0 = 128 - r0
        _OSEGS.append([(s0, r0, 0, n0), (s0 + 1, 0, n0, HS - n0)])


def _chunks(lo, hi, align=512):
    """Split [lo, hi) at multiples of `align`."""
    out = []
    a = lo
    while a < hi:
        b = min(hi, (a // align + 1) * align)
        out.append((a, b))
        a = b
    return out


def build_nc():
    nc = bacc.Bacc()
    xT_d = nc.dram_tensor("xT", [C, T], F32R, kind="ExternalInput")
    wqk_d = nc.dram_tensor("wqk", [C, H, 2 * HS], F32R, kind="ExternalInput")
    wv_d = nc.dram_tensor("wv", [C, C], F32R, kind="ExternalInput")
    wpT_d = nc.dram_tensor("wpT", [C, C], BF16, kind="ExternalInput")
    mk_d = nc.dram_tensor("mk", [128, 128], BF16, kind="ExternalInput")
    bsel_d = nc.dram_tensor("bsel", [HS + 1, HS], F32R, kind="ExternalInput")
    bqk_d = nc.dram_tensor("bqk", [HS, 16], F32, kind="ExternalInput")
    bv_d = nc.dram_tensor("bv", [128, C], F32, kind="ExternalInput")
    bo_d = nc.dram_tensor("bo", [128, C], F32, kind="ExternalInput")
    out = nc.dram_tensor("out", [T, C], F32, kind="ExternalOutput")

    xT_r = xT_d.rearrange("(k p) t -> p k t", p=128)
    wqk_r = wqk_d.rearrange("(k p) h c -> p k h c", p=128)
    wv_r = wv_d.rearrange("(k p) c -> p k c", p=128)
    wpT_r = wpT_d.rearrange("(k p) c -> p k c", p=128)
    out_r = out.rearrange("(a p) c -> p a c", p=128)

    with tile.TileContext(nc) as tc, ExitStack() as ctx:
        consts = ctx.enter_context(tc.tile_pool(name="consts", bufs=1))
        mk_sb = consts.tile([128, 128], BF16, tag="mk")
        bs_sb = consts.tile([HS + 1, HS], F32R, tag="bs")
        bqk_sb = consts.tile([HS, 16], F32, tag="bqk")
        bv_sb = consts.tile([128, C], F32, tag="bv")
        bo_sb = consts.tile([128, C], F32, tag="bo")
        wpT_sb = consts.tile([128, KT, C], BF16, tag="wpT")
        one_bf = consts.tile([128, 1], BF16, tag="one")
        nc.vector.memset(one_bf[:], 1.0)

        xTp = ctx.enter_context(tc.tile_pool(name="xT", bufs=1))
        xT = xTp.tile([128, KT, T], F32R, tag="xT")
        Vp = ctx.enter_context(tc.tile_pool(name="V", bufs=1))
        V = Vp.tile([128, TT, H, HS + 1], BF16, tag="V")
        ostp = ctx.enter_context(tc.tile_pool(name="ost", bufs=1))
        Ost = ostp.tile([128, KT, T], BF16, tag="ost")

        with tc.tile_pool(name="wv", bufs=2) as wvp, \
             tc.tile_pool(name="wqk", bufs=2) as wqkp, \
             tc.tile_pool(name="qk", bufs=4) as qkp, \
             tc.tile_pool(name="pt", bufs=2) as ptp, \
             tc.tile_pool(name="lt", bufs=2) as ltp, \
             tc.tile_pool(name="rc", bufs=2) as rcp, \
             tc.tile_pool(name="stg", bufs=2) as stgp, \
             tc.tile_pool(name="ob", bufs=3) as obp, \
             tc.tile_pool(name="pj", bufs=2, space="PSUM") as pjp, \
             tc.tile_pool(name="sp", bufs=2, space="PSUM") as spp, \
             tc.tile_pool(name="op", bufs=2, space="PSUM") as opp:

            # ---- input DMAs, ordered for the pipeline ----
            # small consts + the first weight slices first, then x^T in
            # 512-column chunks so V/QK projections start early.
            nc.sync.dma_start(mk_sb[:], mk_d[:, :])
            nc.sync.dma_start(bs_sb[:], bsel_d[:, :])
            nc.sync.dma_start(bqk_sb[:], bqk_d[:, :])
            nc.sync.dma_start(bv_sb[:], bv_d[:, :])
            wv_sb = [wvp.tile([128, KT, 4 * HS], F32R, tag="wv",
                              name=f"wv{g}") for g in range(2)]
            nc.sync.dma_start(wv_sb[0][:], wv_r[:, :, 0:4 * HS])
            wqk_sb = {0: wqkp.tile([128, KT, 2 * HS], F32R, tag="wqk",
                                   name="wqk0")}
            nc.sync.dma_start(wqk_sb[0][:], wqk_r[:, :, 0, :])
            for chs, che in ((0, 512), (512, 1024), (1024, 1536), (1536, 2048)):
                nc.sync.dma_start(xT[:, :, chs:che], xT_r[:, :, chs:che])
            nc.sync.dma_start(wv_sb[1][:], wv_r[:, :, 4 * HS:8 * HS])
            wqk_sb[1] = wqkp.tile([128, KT, 2 * HS], F32R, tag="wqk",
                                  name="wqk1")
            nc.sync.dma_start(wqk_sb[1][:], wqk_r[:, :, 1, :])
            nc.sync.dma_start(bo_sb[:], bo_d[:, :])
            nc.sync.dma_start(wpT_sb[:], wpT_r[:, :, :])

            # ones columns for the softmax denominator
            nc.vector.tensor_copy(V[:, :, :, HS:HS + 1],
                                  one_bf[:].to_broadcast([128, TT, H, 1]))

            def qk_proj_chunk(h, tc4):
                """Q^T/K^T projection for head h, t-columns [512*tc4, +512)."""
                for mc in range(2):          # 0 = q, 1 = k
                    pj = pjp.tile([128, 512], F32, tag="pj", name="pj")
                    for kc in range(KT):
                        nc.tensor.matmul(
                            pj[0:HS, 0:512],
                            wqk_sb[h][:, kc, mc * HS:(mc + 1) * HS],
                            xT[:, kc, tc4 * 512:(tc4 + 1) * 512],
                            start=(kc == 0), stop=(kc == KT - 1))
                    m_col = h + (0 if mc == 0 else 8)
                    nc.vector.tensor_tensor(
                        qkh[h][mc][0:HS, tc4 * 512:(tc4 + 1) * 512],
                        pj[0:HS, 0:512],
                        bqk_sb[:, m_col:m_col + 1].to_broadcast([HS, 512]),
                        ADD)

            # qk tiles for head h and h+1 live concurrently (pool bufs=4)
            qkh = {}

            def alloc_qk(h):
                qkh[h] = [qkp.tile([HS, T], BF16, tag="qk", name=f"qk{h}_{i}")
                          for i in range(2)]

            # ---- V projection (all heads) + head-0 QK proj, pipelined with
            # the x^T chunk DMAs ----
            alloc_qk(0)
            for ch in range(4):
                for g in range(2):
                    for tt in range(4 * ch, 4 * ch + 4):
                        vps = pjp.tile([128, 512], F32, tag="pj", name="vps")
                        for kc in range(KT):
                            nc.tensor.matmul(
                                vps[:, 0:4 * HS],
                                xT[:, kc, tt * 128:(tt + 1) * 128],
                                wv_sb[g][:, kc, :],
                                start=(kc == 0), stop=(kc == KT - 1))
                        nc.vector.tensor_tensor(
                            V[:, tt, 4 * g:4 * g + 4, 0:HS],
                            vps[:, 0:4 * HS].rearrange("p (h d) -> p h d",
                                                       d=HS),
                            bv_sb[:, 4 * HS * g:4 * HS * (g + 1)]
                                .rearrange("p (h d) -> p h d", d=HS),
                            ADD)
                qk_proj_chunk(0, ch)

            def attn_half(h, ihalf, after_jt0=None):
                """S^T/P/O^T accumulation for rows [1024*ihalf, +1024)."""
                qT, kT = qkh[h]
                ibase = 1024 * ihalf
                iend = ibase + 1024
                njt = 8 * (ihalf + 1)
                O_ps = opp.tile([128, 1024], F32, tag="O", name="O_ps")
                for jt in range(njt):
                    j0 = 128 * jt
                    i0 = max(j0, ibase)
                    ilen = iend - i0
                    P = ptp.tile([128, 1024], BF16, tag="P", name="P")
                    for (ra, rb) in _chunks(0, ilen):
                        S = spp.tile([128, 512], F32, tag="S", name="S")
                        nc.tensor.matmul(S[:, 0:rb - ra],
                                         kT[0:HS, j0:j0 + 128],
                                         qT[0:HS, i0 + ra:i0 + rb],
                                         start=True, stop=True)
                        nc.scalar.activation(P[:, ra:rb], S[:, 0:rb - ra],
                                             EXP)
                    if j0 >= ibase:
                        nc.gpsimd.tensor_tensor(P[:, 0:128], P[:, 0:128],
                                                mk_sb[:], MULT)
                    for (a, b) in _chunks(i0, iend):
                        ci = a // 512
                        last_jt = min(4 * ci + 3, njt - 1)
                        nc.tensor.matmul(
                            O_ps[0:HS + 1, a - ibase:b - ibase],
                            V[:, jt, h, :],
                            P[:, a - i0:b - i0],
                            start=(jt == 0), stop=(jt == last_jt))
                    if jt == 0 and after_jt0 is not None:
                        after_jt0()
                return O_ps

            def epilogue(h, ihalf, O_ps):
                """Normalize O^T by the denominator row (row HS of O_ps) and
                write the bf16 result into the 128-row-aligned stripes."""
                ibase = 1024 * ihalf
                iend = ibase + 1024
                lt = ltp.tile([HS + 1, 1024], F32R, tag="lt", name="lt")
                nc.vector.tensor_copy(lt[:], O_ps[0:HS + 1, :])
                Lp = opp.tile([128, 1024], F32, tag="O", name="Lp")
                R = rcp.tile([HS, 1024], F32, tag="R", name="R")
                for (ra, rb) in ((0, 512), (512, 1024)):
                    nc.tensor.matmul(Lp[0:HS, ra:rb], bs_sb[:, :],
                                     lt[:, ra:rb], start=True, stop=True)
                    nc.vector.reciprocal_approx_fast(R[:, ra:rb],
                                                     Lp[0:HS, ra:rb])
                stg = stgp.tile([HS, 1024], BF16, tag="stg", name="stg")
                nc.gpsimd.tensor_tensor(stg[:], lt[0:HS, :], R[:], MULT)
                # repack into the 128-row-aligned stripes via DMA (engines
                # cannot cross partition quadrants; DMA can)
                for (s, r0, d0, ln) in _OSEGS[h]:
                    nc.sync.dma_start(Ost[r0:r0 + ln, s, ibase:iend],
                                      stg[d0:d0 + ln, :])

            # ---- per-head attention ----
            for h in range(H):
                if h + 2 < H:
                    wqk_sb[h + 2] = wqkp.tile([128, KT, 2 * HS], F32R,
                                              tag="wqk", name=f"wqk{h + 2}")
                    nc.sync.dma_start(wqk_sb[h + 2][:], wqk_r[:, :, h + 2, :])

                O0 = attn_half(h, 0)
                O1 = attn_half(h, 1,
                               after_jt0=lambda: epilogue(h, 0, O0))
                if h + 1 < H:
                    alloc_qk(h + 1)
                    for tc4 in range(4):
                        qk_proj_chunk(h + 1, tc4)
                epilogue(h, 1, O1)

            # ---- output projection, from SBUF, K=128 stripes ----
            for tt in range(TT):
                cps = opp.tile([128, 1024], F32, tag="O", name="cps")
                for (a, b) in ((0, 512), (512, C)):
                    for kc in range(KT):
                        nc.tensor.matmul(cps[:, a:b],
                                         Ost[:, kc, tt * 128:(tt + 1) * 128],
                                         wpT_sb[:, kc, a:b],
                                         start=(kc == 0), stop=(kc == KT - 1))
                o_sb = obp.tile([128, C], F32, tag="o", name="o_sb")
                nc.vector.tensor_tensor(o_sb[:], cps[:, 0:C], bo_sb[:], ADD)
                nc.gpsimd.dma_start(out_r[:, tt], o_sb[:])

    nc.finalize()
    return nc


_NC_CACHE = {}


def _get_nc():
    if "nc" not in _NC_CACHE:
        _NC_CACHE["nc"] = build_nc()
    return _NC_CACHE["nc"]


def _make_consts(w_attn, b_attn, w_proj, b_proj):
    s = 1.0 / math.sqrt(HS)
    waT = np.ascontiguousarray(w_attn.T)          # [C, 3C]
    wqk = np.empty((C, H, 2 * HS), dtype=np.float32)
    for h in range(H):
        wqk[:, h, 0:HS] = waT[:, h * HS:(h + 1) * HS] * s
        wqk[:, h, HS:2 * HS] = waT[:, C + h * HS:C + (h + 1) * HS]
    wv = np.ascontiguousarray(waT[:, 2 * C:3 * C])
    wpT = np.ascontiguousarray(w_proj.T).astype(BF16_NP)
    bqk = np.empty((HS, 16), dtype=np.float32)
    for m in range(8):
        bqk[:, m] = b_attn[m * HS:(m + 1) * HS] * s
    for m in range(8):
        bqk[:, 8 + m] = b_attn[C + m * HS:C + (m + 1) * HS]
    bv = np.ascontiguousarray(
        np.broadcast_to(b_attn[2 * C:3 * C], (128, C)).astype(np.float32))
    bo = np.ascontiguousarray(
        np.broadcast_to(b_proj, (128, C)).astype(np.float32))
    mk = np.triu(np.ones((128, 128), dtype=np.float32)).astype(BF16_NP)
    bsel = np.zeros((HS + 1, HS), dtype=np.float32)
    bsel[HS, :] = 1.0
    return wqk, wv, wpT, bqk, bv, bo, mk, bsel


def kernel(x, w_attn, b_attn, w_proj, b_proj, _want_results=False, **run_kwargs):
    x = np.asarray(x, dtype=np.float32)
    w_attn = np.asarray(w_attn, dtype=np.float32)
    b_attn = np.asarray(b_attn, dtype=np.float32)
    w_proj = np.asarray(w_proj, dtype=np.float32)
    b_proj = np.asarray(b_proj, dtype=np.float32)

    wqk, wv, wpT, bqk, bv, bo, mk, bsel = _make_consts(
        w_attn, b_attn, w_proj, b_proj)

    nc = _get_nc()
    common = dict(wqk=wqk, wv=wv, wpT=wpT, bqk=bqk, bv=bv, bo=bo, mk=mk,
                  bsel=bsel)
    in_maps = [dict(xT=np.ascontiguousarray(x[c].T), **common)
               for c in range(NCORES)]
    res = run_bass_kernel_spmd(nc, in_maps, core_ids=list(range(NCORES)),
                               **run_kwargs)
    out = np.stack([res.results[c]["out"] for c in range(NCORES)], axis=0)
    if _want_results:
        return out, res
    return out


if __name__ == "__main__":
    rng = np.random.default_rng(0)
    x = rng.standard_normal((B, T, C), dtype=np.float32)
    w_attn = rng.standard_normal((3 * C, C), dtype=np.float32) / math.sqrt(C)
    b_attn = rng.standard_normal(3 * C).astype(np.float32) * 0.02
    w_proj = rng.standard_normal((C, C), dtype=np.float32) / math.sqrt(C)
    b_proj = rng.standard_normal(C).astype(np.float32) * 0.02
    o = kernel(x, w_attn, b_attn, w_proj, b_proj)
    print("out", o.shape, o.dtype, float(np.abs(o).mean()))


# revision 10
# speedup vs baseline: 1.6376x; 1.0481x over previous
"""Causal multi-head attention block (B=8, T=2048, C=768, H=8) on 8 trn2 cores.

Sharding: data-parallel over batch — one batch element per NeuronCore, weights
replicated, no collectives.

Host prep: x, w_attn, w_proj are pre-transposed on the host (numpy), the
1/sqrt(hs) logit scale is folded into the Q weights/bias, and w_proj^T is cast
to bf16 — so the device kernel has no transpose phase at all.

Per-core algorithm:
  Load x^T / w^T slices straight into SBUF (weights streamed per-head).
  V = x @ w_v^T + b_v for all 8 heads in natural [t, d] layout (bf16, with a
  ones column per head for the softmax denominator).
  Per head: Q^T/K^T = w x^T + b in head-aligned [d, t] layout (bf16);
  causal attention in S^T layout: S^T[j, i] matmul (f32 PSUM) -> P = exp(S^T)
  on ACT (bf16 out) -> diagonal-block mask multiply on gpsimd -> O^T[d, i]
  (+ denominator row l) accumulated in PSUM over j-tiles via lhsT=[V|1]
  (bf16).  Epilogue (emission deferred so it never head-of-line-blocks the
  tensor queue): copy O_ps to SBUF on gpsimd, broadcast l across partitions
  with a selector matmul, fast approximate reciprocal on DVE, then normalize
  on gpsimd straight into a 128-row-aligned resident O^T stripe tile (bf16,
  no DRAM spill).
  Output projection from SBUF per t-tile with K=128 stripe contraction:
  out = O^T.T @ w_proj^T + b_proj, written [t, c].
"""

import math
import os
import sys
from contextlib import ExitStack

for _p in ("/opt/trn_rl_repo", "/root/.axon_site/_ro/trn_rl_repo"):
    if os.path.isdir(_p) and _p not in sys.path:
        sys.path.append(_p)

import numpy as np
import ml_dtypes

import concourse.bass as bass  # noqa: F401  (import keeps bass registered)
from concourse import bacc
import concourse.mybir as mybir
import concourse.tile as tile
from concourse.bass_utils import run_bass_kernel_spmd

F32 = mybir.dt.float32
F32R = mybir.dt.float32r
BF16 = mybir.dt.bfloat16
EXP = mybir.ActivationFunctionType.Exp
ADD = mybir.AluOpType.add
MULT = mybir.AluOpType.mult

B, T, C, H, HS = 8, 2048, 768, 8, 96
KT = C // 128        # 6 contraction tiles of 128
TT = T // 128        # 16 t-tiles of 128
NCORES = 8
BF16_NP = ml_dtypes.bfloat16

# head h occupies rows 96h..96h+95 of the packed [768, T] O^T; as 6 stripes of
# 128 partitions each head maps to 1-2 (stripe, row0, d0, length) segments
_OSEGS = []
for _h in range(H):
    g0 = _h * HS
    s0, r0 = g0 // 128, g0 % 128
    if r0 + HS <= 128:
        _OSEGS.append([(s0, r0, 0, HS)])
    else:
        n0 = 128 - r0
        _OSEGS.append([(s0, r0, 0, n0), (s0 + 1, 0, n0, HS - n0)])


def _chunks(lo, hi, align=512):
    """Split [lo, hi) at multiples of `align`."""
    out = []
    a = lo
    while a < hi:
        b = min(hi, (a // align + 1) * align)
        out.append((a, b))
        a = b
    return out


def build_nc():
    nc = bacc.Bacc()
    xT_d = nc.dram_tensor("xT", [C, T], F32R, kind="ExternalInput")
    wqk_d = nc.dram_tensor("wqk", [C, H, 2 * HS], F32R, kind="ExternalInput")
    wv_d = nc.dram_tensor("wv", [C, C], F32R, kind="ExternalInput")
    wpT_d = nc.dram_tensor("wpT", [C, C], BF16, kind="ExternalInput")
    mk_d = nc.dram_tensor("mk", [128, 128], BF16, kind="ExternalInput")
    bsel_d = nc.dram_tensor("bsel", [HS + 1, HS], F32R, kind="ExternalInput")
    bqk_d = nc.dram_tensor("bqk", [HS, 16], F32, kind="ExternalInput")
    bv_d = nc.dram_tensor("bv", [128, C], F32, kind="ExternalInput")
    bo_d = nc.dram_tensor("bo", [128, C], F32, kind="ExternalInput")
    out = nc.dram_tensor("out", [T, C], F32, kind="ExternalOutput")

    xT_r = xT_d.rearrange("(k p) t -> p k t", p=128)
    wqk_r = wqk_d.rearrange("(k p) h c -> p k h c", p=128)
    wv_r = wv_d.rearrange("(k p) c -> p k c", p=128)
    wpT_r = wpT_d.rearrange("(k p) c -> p k c", p=128)
    out_r2 = out.rearrange("(g a p) c -> p g a c", a=2, p=128)

    with tile.TileContext(nc) as tc, ExitStack() as ctx:
        consts = ctx.enter_context(tc.tile_pool(name="consts", bufs=1))
        mk_sb = consts.tile([128, 128], BF16, tag="mk")
        bs_sb = consts.tile([HS + 1, HS], F32R, tag="bs")
        bqk_sb = consts.tile([HS, 16], F32, tag="bqk")
        bv_sb = consts.tile([128, C], F32, tag="bv")
        bo_sb = consts.tile([128, C], F32, tag="bo")
        wpT_sb = consts.tile([128, KT, C], BF16, tag="wpT")
        one_bf = consts.tile([128, 1], BF16, tag="one")
        nc.vector.memset(one_bf[:], 1.0)

        xTp = ctx.enter_context(tc.tile_pool(name="xT", bufs=1))
        xT = xTp.tile([128, KT, T], F32R, tag="xT")
        Vp = ctx.enter_context(tc.tile_pool(name="V", bufs=1))
        V = Vp.tile([128, TT, H, HS + 1], BF16, tag="V")
        ostp = ctx.enter_context(tc.tile_pool(name="ost", bufs=1))
        Ost = ostp.tile([128, KT, T], BF16, tag="ost")

        with tc.tile_pool(name="wv", bufs=2) as wvp, \
             tc.tile_pool(name="wqk", bufs=2) as wqkp, \
             tc.tile_pool(name="qk", bufs=4) as qkp, \
             tc.tile_pool(name="pt", bufs=2) as ptp, \
             tc.tile_pool(name="lt", bufs=2) as ltp, \
             tc.tile_pool(name="rc", bufs=2) as rcp, \
             tc.tile_pool(name="stg", bufs=2) as stgp, \
             tc.tile_pool(name="ob", bufs=3) as obp, \
             tc.tile_pool(name="pj", bufs=2, space="PSUM") as pjp, \
             tc.tile_pool(name="sp", bufs=2, space="PSUM") as spp, \
             tc.tile_pool(name="op", bufs=2, space="PSUM") as opp:

            # ---- input DMAs, ordered for the pipeline ----
            # the first V-projection needs bv + wv0 + the first x^T columns;
            # wv1 right behind so the second head-group is never starved;
            # everything else ordered by first use.
            nc.sync.dma_start(bv_sb[:], bv_d[:, :])
            wv_sb = [wvp.tile([128, KT, 4 * HS], F32R, tag="wv",
                              name=f"wv{g}") for g in range(2)]
            nc.sync.dma_start(wv_sb[0][:], wv_r[:, :, 0:4 * HS])
            nc.sync.dma_start(xT[:, :, 0:256], xT_r[:, :, 0:256])
            nc.sync.dma_start(xT[:, :, 256:512], xT_r[:, :, 256:512])
            nc.sync.dma_start(wv_sb[1][:], wv_r[:, :, 4 * HS:8 * HS])
            wqk_sb = {0: wqkp.tile([128, KT, 2 * HS], F32R, tag="wqk",
                                   name="wqk0")}
            nc.sync.dma_start(wqk_sb[0][:], wqk_r[:, :, 0, :])
            nc.sync.dma_start(bqk_sb[:], bqk_d[:, :])
            nc.sync.dma_start(xT[:, :, 512:1024], xT_r[:, :, 512:1024])
            nc.sync.dma_start(mk_sb[:], mk_d[:, :])
            nc.sync.dma_start(xT[:, :, 1024:1536], xT_r[:, :, 1024:1536])
            nc.sync.dma_start(xT[:, :, 1536:2048], xT_r[:, :, 1536:2048])
            nc.sync.dma_start(bs_sb[:], bsel_d[:, :])
            wqk_sb[1] = wqkp.tile([128, KT, 2 * HS], F32R, tag="wqk",
                                  name="wqk1")
            nc.sync.dma_start(wqk_sb[1][:], wqk_r[:, :, 1, :])
            nc.sync.dma_start(bo_sb[:], bo_d[:, :])
            nc.sync.dma_start(wpT_sb[:], wpT_r[:, :, :])

            # ones columns for the softmax denominator
            nc.vector.tensor_copy(V[:, :, :, HS:HS + 1],
                                  one_bf[:].to_broadcast([128, TT, H, 1]))

            def qk_proj_chunk(h, tc4):
                """Q^T/K^T projection for head h, t-columns [512*tc4, +512)."""
                for mc in range(2):          # 0 = q, 1 = k
                    pj = pjp.tile([128, 512], F32, tag="pj", name="pj")
                    for kc in range(KT):
                        nc.tensor.matmul(
                            pj[0:HS, 0:512],
                            wqk_sb[h][:, kc, mc * HS:(mc + 1) * HS],
                            xT[:, kc, tc4 * 512:(tc4 + 1) * 512],
                            start=(kc == 0), stop=(kc == KT - 1))
                    m_col = h + (0 if mc == 0 else 8)
                    nc.vector.tensor_tensor(
                        qkh[h][mc][0:HS, tc4 * 512:(tc4 + 1) * 512],
                        pj[0:HS, 0:512],
                        bqk_sb[:, m_col:m_col + 1].to_broadcast([HS, 512]),
                        ADD)

            # qk tiles for head h and h+1 live concurrently (pool bufs=4)
            qkh = {}

            def alloc_qk(h):
                qkh[h] = [qkp.tile([HS, T], BF16, tag="qk", name=f"qk{h}_{i}")
                          for i in range(2)]

            # ---- V projection (all heads) + head-0 QK proj, pipelined with
            # the x^T chunk DMAs ----
            alloc_qk(0)
            for ch in range(4):
                for g in range(2):
                    for tt in range(4 * ch, 4 * ch + 4):
                        vps = pjp.tile([128, 512], F32, tag="pj", name="vps")
                        for kc in range(KT):
                            nc.tensor.matmul(
                                vps[:, 0:4 * HS],
                                xT[:, kc, tt * 128:(tt + 1) * 128],
                                wv_sb[g][:, kc, :],
                                start=(kc == 0), stop=(kc == KT - 1))
                        nc.vector.tensor_tensor(
                            V[:, tt, 4 * g:4 * g + 4, 0:HS],
                            vps[:, 0:4 * HS].rearrange("p (h d) -> p h d",
                                                       d=HS),
                            bv_sb[:, 4 * HS * g:4 * HS * (g + 1)]
                                .rearrange("p (h d) -> p h d", d=HS),
                            ADD)
                qk_proj_chunk(0, ch)

            def attn_half(h, ihalf, after_jt0=None):
                """S^T/P/O^T accumulation for rows [1024*ihalf, +1024)."""
                qT, kT = qkh[h]
                ibase = 1024 * ihalf
                iend = ibase + 1024
                njt = 8 * (ihalf + 1)
                O_ps = opp.tile([128, 1024], F32, tag="O", name="O_ps")
                for jt in range(njt):
                    j0 = 128 * jt
                    i0 = max(j0, ibase)
                    ilen = iend - i0
                    P = ptp.tile([128, 1024], BF16, tag="P", name="P")
                    for (ra, rb) in _chunks(0, ilen):
                        S = spp.tile([128, 512], F32, tag="S", name="S")
                        nc.tensor.matmul(S[:, 0:rb - ra],
                                         kT[0:HS, j0:j0 + 128],
                                         qT[0:HS, i0 + ra:i0 + rb],
                                         start=True, stop=True)
                        nc.scalar.activation(P[:, ra:rb], S[:, 0:rb - ra],
                                             EXP)
                    if j0 >= ibase:
                        nc.gpsimd.tensor_tensor(P[:, 0:128], P[:, 0:128],
                                                mk_sb[:], MULT)
                    for (a, b) in _chunks(i0, iend):
                        ci = a // 512
                        last_jt = min(4 * ci + 3, njt - 1)
                        nc.tensor.matmul(
                            O_ps[0:HS + 1, a - ibase:b - ibase],
                            V[:, jt, h, :],
                            P[:, a - i0:b - i0],
                            start=(jt == 0), stop=(jt == last_jt))
                    if jt == 0 and after_jt0 is not None:
                        after_jt0()
                return O_ps

            def epilogue(h, ihalf, O_ps):
                """Normalize O^T by the denominator row (row HS of O_ps) and
                write the bf16 result into the 128-row-aligned stripes."""
                ibase = 1024 * ihalf
                iend = ibase + 1024
                lt = ltp.tile([HS + 1, 1024], F32R, tag="lt", name="lt")
                nc.vector.tensor_copy(lt[:], O_ps[0:HS + 1, :])
                Lp = opp.tile([128, 1024], F32, tag="O", name="Lp")
                R = rcp.tile([HS, 1024], F32, tag="R", name="R")
                for (ra, rb) in ((0, 512), (512, 1024)):
                    nc.tensor.matmul(Lp[0:HS, ra:rb], bs_sb[:, :],
                                     lt[:, ra:rb], start=True, stop=True)
                    nc.vector.reciprocal_approx_fast(R[:, ra:rb],
                                                     Lp[0:HS, ra:rb])
                stg = stgp.tile([HS, 1024], BF16, tag="stg", name="stg")
                nc.gpsimd.tensor_tensor(stg[:], lt[0:HS, :], R[:], MULT)
                # repack into the 128-row-aligned stripes via DMA (engines
                # cannot cross partition quadrants; DMA can)
                for (s, r0, d0, ln) in _OSEGS[h]:
                    nc.sync.dma_start(Ost[r0:r0 + ln, s, ibase:iend],
                                      stg[d0:d0 + ln, :])

            # ---- per-head attention ----
            for h in range(H):
                if h + 2 < H:
                    wqk_sb[h + 2] = wqkp.tile([128, KT, 2 * HS], F32R,
                                              tag="wqk", name=f"wqk{h + 2}")
                    nc.sync.dma_start(wqk_sb[h + 2][:], wqk_r[:, :, h + 2, :])

                O0 = attn_half(h, 0)
                O1 = attn_half(h, 1,
                               after_jt0=lambda: epilogue(h, 0, O0))
                if h + 1 < H:
                    alloc_qk(h + 1)
                    for tc4 in range(4):
                        qk_proj_chunk(h + 1, tc4)
                epilogue(h, 1, O1)

            # ---- output projection, from SBUF, K=128 stripes ----
            # two t-tiles per staging tile -> 8 fat out DMAs (fewer
            # descriptors + completion semaphores on the tail)
            for tg in range(TT // 2):
                o_sb = obp.tile([128, 2, C], F32, tag="o", name="o_sb")
                for ta in range(2):
                    tt = 2 * tg + ta
                    cps = opp.tile([128, 1024], F32, tag="O", name="cps")
                    for (a, b) in ((0, 512), (512, C)):
                        for kc in range(KT):
                            nc.tensor.matmul(
                                cps[:, a:b],
                                Ost[:, kc, tt * 128:(tt + 1) * 128],
                                wpT_sb[:, kc, a:b],
                                start=(kc == 0), stop=(kc == KT - 1))
                    nc.vector.tensor_tensor(o_sb[:, ta, :], cps[:, 0:C],
                                            bo_sb[:], ADD)
                nc.gpsimd.dma_start(out_r2[:, tg], o_sb[:])

    nc.finalize()
    return nc


_NC_CACHE = {}


def _get_nc():
    if "nc" not in _NC_CACHE:
        _NC_CACHE["nc"] = build_nc()
    return _NC_CACHE["nc"]


def _make_consts(w_attn, b_attn, w_proj, b_proj):
    s = 1.0 / math.sqrt(HS)
    waT = np.ascontiguousarray(w_attn.T)          # [C, 3C]
    wqk = np.empty((C, H, 2 * HS), dtype=np.float32)
    for h in range(H):
        wqk[:, h, 0:HS] = waT[:, h * HS:(h + 1) * HS] * s
        wqk[:, h, HS:2 * HS] = waT[:, C + h * HS:C + (h + 1) * HS]
    wv = np.ascontiguousarray(waT[:, 2 * C:3 * C])
    wpT = np.ascontiguousarray(w_proj.T).astype(BF16_NP)
    bqk = np.empty((HS, 16), dtype=np.float32)
    for m in range(8):
        bqk[:, m] = b_attn[m * HS:(m + 1) * HS] * s
    for m in range(8):
        bqk[:, 8 + m] = b_attn[C + m * HS:C + (m + 1) * HS]
    bv = np.ascontiguousarray(
        np.broadcast_to(b_attn[2 * C:3 * C], (128, C)).astype(np.float32))
    bo = np.ascontiguousarray(
        np.broadcast_to(b_proj, (128, C)).astype(np.float32))
    mk = np.triu(np.ones((128, 128), dtype=np.float32)).astype(BF16_NP)
    bsel = np.zeros((HS + 1, HS), dtype=np.float32)
    bsel[HS, :] = 1.0
    return wqk, wv, wpT, bqk, bv, bo, mk, bsel


def kernel(x, w_attn, b_attn, w_proj, b_proj, _want_results=False, **run_kwargs):
    x = np.asarray(x, dtype=np.float32)
    w_attn = np.asarray(w_attn, dtype=np.float32)
    b_attn = np.asarray(b_attn, dtype=np.float32)
    w_proj = np.asarray(w_proj, dtype=np.float32)
    b_proj = np.asarray(b_proj, dtype=np.float32)

    wqk, wv, wpT, bqk, bv, bo, mk, bsel = _make_consts(
        w_attn, b_attn, w_proj, b_proj)

    nc = _get_nc()
    common = dict(wqk=wqk, wv=wv, wpT=wpT, bqk=bqk, bv=bv, bo=bo, mk=mk,
                  bsel=bsel)
    in_maps = [dict(xT=np.ascontiguousarray(x[c].T), **common)
               for c in range(NCORES)]
    res = run_bass_kernel_spmd(nc, in_maps, core_ids=list(range(NCORES)),
                               **run_kwargs)
    out = np.stack([res.results[c]["out"] for c in range(NCORES)], axis=0)
    if _want_results:
        return out, res
    return out


if __name__ == "__main__":
    rng = np.random.default_rng(0)
    x = rng.standard_normal((B, T, C), dtype=np.float32)
    w_attn = rng.standard_normal((3 * C, C), dtype=np.float32) / math.sqrt(C)
    b_attn = rng.standard_normal(3 * C).astype(np.float32) * 0.02
    w_proj = rng.standard_normal((C, C), dtype=np.float32) / math.sqrt(C)
    b_proj = rng.standard_normal(C).astype(np.float32) * 0.02
    o = kernel(x, w_attn, b_attn, w_proj, b_proj)
    print("out", o.shape, o.dtype, float(np.abs(o).mean()))
